# revision 13
# baseline (speedup 1.0000x reference)
"""AttentionBlock (GroupNorm -> 1x1 qkv -> 4-head attention -> 1x1 proj -> residual)
on 8 trn2 NeuronCores, data-parallel over the batch dim (B=8, one element/core).

Layout per core: channel-major [C=512, N=1024] as 4 SBUF tiles of [128, 1024].
V is computed spatial-major directly from the qkv matmul so attention needs no
transposes:
  ST[j,i] = sum_d k[d,j] q[d,i]   (K-tile stationary)
  p~T[j,i] = exp(scale*ST)        (ScalarE, PSUM->SBUF)
  rowsum[i] = ones^T @ p~T        (PE)
  PV[d,i] = sum_j v_sp[j,d] p~T[j,i]   -> channel-major attention output
  out = PV * (1/rowsum broadcast)      (softmax divide deferred past PV)
v-bias is folded into proj bias on the host (softmax rows sum to 1).

v2 schedule: residual taken from the bf16 x copy (no f32 x load), GN rstd via
ln/exp (one ACT table for the whole kernel), q0/k0 qkv matmuls wave-interleaved
with groupnorm, qkv bias adds on DVE (q) / GpSimd (k), v casts on GpSimd,
all-bf16 softmax reduction tree, proj in two PSUM waves starting as heads
complete.
"""

import numpy as np

B, C, H, W = 8, 512, 32, 32
N = H * W  # 1024
NUM_HEADS = 4
HEAD_DIM = C // NUM_HEADS  # 128
NUM_GROUPS = 32
GROUP_CH = C // NUM_GROUPS  # 16
EPS = 1e-5
NT = C // 128  # 4 channel tiles
NO_QK = 8  # q,k output tiles (1024 channels)
SCALE = 1.0 / float(np.sqrt(HEAD_DIM))
N_CORES = 8


def build_bass():
    import concourse.bacc as bacc
    import concourse.tile as tile
    from concourse import mybir

    f32 = mybir.dt.float32
    bf16 = mybir.dt.bfloat16
    Act = mybir.ActivationFunctionType
    Alu = mybir.AluOpType
    Ax = mybir.AxisListType

    nc = bacc.Bacc("TRN2", target_bir_lowering=False, debug=False,
                   num_devices=N_CORES)

    d_xb = nc.declare_dram_parameter("xb", [C, N], bf16, isOutput=False)
    d_wt = nc.declare_dram_parameter("qkv_wt", [C, 3 * C], bf16, isOutput=False)
    d_pwt = nc.declare_dram_parameter("proj_wt", [C, C], bf16, isOutput=False)
    d_cv = nc.declare_dram_parameter("cvec", [128, 28], f32, isOutput=False)
    d_selT = nc.declare_dram_parameter("selT", [8, 128], f32, isOutput=False)
    d_ones = nc.declare_dram_parameter("ones", [128, 1], bf16, isOutput=False)
    d_out = nc.declare_dram_parameter("out", [C, N], f32, isOutput=True)

    with tile.TileContext(nc) as tc:
        with (
            tc.tile_pool(name="persist", bufs=1) as pp,
            tc.tile_pool(name="pt", bufs=20) as p_pt,
            tc.tile_pool(name="rsum", bufs=5) as p_rs,
            tc.tile_pool(name="outp", bufs=2) as p_out,
            tc.tile_pool(name="small", bufs=1) as ps,
            tc.tile_pool(name="psum", bufs=2, space="PSUM") as pm,
        ):
            # ---- tiny constants + ACT table warm (exp then ln narrows the
            # activation table to natural_log_exp_and_others, which also
            # serves Square/Identity: one table load for the whole kernel,
            # during the DMA window).
            warm = ps.tile([128, 512], bf16, tag="warm", name="warm")
            nc.vector.memset(warm[:], 0.5)
            epsv = ps.tile([8, 2], f32, tag="epsv", name="epsv")
            nc.vector.memset(epsv[:], EPS)
            tw = ps.tile([1, 2], f32, tag="tw", name="tw")
            nc.scalar.activation(tw[:, 0:1], epsv[0:1, 0:1], Act.Exp)
            nc.scalar.activation(tw[:, 1:2], epsv[0:1, 1:2], Act.Ln)

            cvec = ps.tile([128, 28], f32, tag="cvec", name="cvec")
            gam, bet, bqk, beff = (cvec[:, 0:4], cvec[:, 4:8],
                                   cvec[:, 8:16], cvec[:, 16:20])
            sel = cvec[:, 20:28]
            selT = ps.tile([8, 128], f32, tag="selT", name="selT")
            ones_r = ps.tile([128, 1], bf16, tag="ones_r", name="ones_r")

            # ---- bulk loads: xb/wt interleaved on the two HWDGE queues so
            # the GN/QKV critical path tensors land first; proj weights last
            # on the gpsimd SWDGE queue.
            xbs, hs, wts, pwts = [], [], [], []
            for t in range(NT):
                xb_t = pp.tile([128, N], bf16, tag=f"xb{t}", name=f"xb{t}")
                xbs.append(xb_t)
            for t in range(NT):
                wt_t = pp.tile([128, 3 * C], bf16, tag=f"wt{t}", name=f"wt{t}")
                wts.append(wt_t)
            for t in range(NT):
                eng = nc.sync if t % 2 == 0 else nc.scalar
                eng.dma_start(xbs[t][:], d_xb[t * 128:(t + 1) * 128, :])
            nc.scalar.dma_start(cvec[:], d_cv[:, :])
            nc.scalar.dma_start(selT[:], d_selT[:, :])
            for t in range(NT):
                eng = nc.sync if t % 2 == 0 else nc.gpsimd
                eng.dma_start(wts[t][:], d_wt[t * 128:(t + 1) * 128, :])
            nc.sync.dma_start(ones_r[:], d_ones[:, :])
            for t in range(NT):
                pwt_t = pp.tile([128, C], bf16, tag=f"pwt{t}", name=f"pwt{t}")
                nc.gpsimd.dma_start(pwt_t[:], d_pwt[t * 128:(t + 1) * 128, :])
                pwts.append(pwt_t)

            # PE warm-up: junk matmul chain (never read) holds the PE p-state
            # up while the first xb tiles stream in.
            junk = pm.tile([128, N], f32, tag="acc", name="junk")

            def junk_mm(n, first=False, last=False):
                for j in range(n):
                    nc.tensor.matmul(junk[0:128, 0:512], warm[:, 0:128],
                                     warm[:, 0:512],
                                     start=(first and j == 0),
                                     stop=(last and j == n - 1),
                                     skip_group_check=True)

            junk_mm(5, first=True, last=True)

            # q0/k0 accumulate across GN tiles (wave A)
            pqq = pm.tile([128, N], f32, tag="acc", name="pqq")
            pqk = pm.tile([128, N], f32, tag="acc", name="pqk")

            # ---- group norm per-tile (groups never cross 128-ch tiles),
            # with the head-0 qkv matmuls riding along as h tiles appear.
            for t in range(NT):
                h_t = pp.tile([128, N], bf16, tag=f"h{t}", name=f"h{t}")
                hs.append(h_t)
            for t in range(NT):
                st_t = ps.tile([128, 2], f32, tag=f"st{t}", name=f"st{t}")
                nc.vector.reduce_sum(st_t[:, 0:1], xbs[t][:], axis=Ax.X)
                nc.scalar.activation(hs[t][:], xbs[t][:], Act.Square,
                                     accum_out=st_t[:, 1:2])
                psg = pm.tile([128, N], f32, tag="ps", name=f"psg{t}")
                nc.tensor.matmul(psg[0:8, 0:2], sel, st_t[:, 0:2],
                                 start=True, stop=True)
                inv_n = 1.0 / float(GROUP_CH * N)
                msr = ps.tile([8, 4], f32, tag=f"msr{t}", name=f"msr{t}")
                nc.scalar.mul(msr[:, 0:1], psg[0:8, 0:1], inv_n)
                nc.scalar.square(msr[:, 3:4], msr[:, 0:1])
                nc.vector.scalar_tensor_tensor(msr[:, 2:3], psg[0:8, 1:2],
                                               inv_n, msr[:, 3:4],
                                               op0=Alu.mult, op1=Alu.subtract)
                # rstd = exp(-0.5*ln(var+eps)) keeps ACT on the exp table
                nc.scalar.activation(msr[:, 3:4], msr[:, 2:3], Act.Ln,
                                     bias=epsv[:, 0:1])
                nc.scalar.activation(msr[:, 1:2], msr[:, 3:4], Act.Exp,
                                     scale=-0.5)
                pse = pm.tile([128, N], f32, tag="ps", name=f"pse{t}")
                nc.tensor.matmul(pse[:, 0:2], selT[:], msr[:, 0:2],
                                 start=True, stop=True)
                ab_t = ps.tile([128, 3], f32, tag=f"ab{t}", name=f"ab{t}")
                nc.vector.tensor_mul(ab_t[:, 0:1], gam[:, t:t + 1], pse[:, 1:2])
                nc.vector.tensor_mul(ab_t[:, 2:3], pse[:, 0:1], ab_t[:, 0:1])
                nc.vector.tensor_sub(ab_t[:, 1:2], bet[:, t:t + 1], ab_t[:, 2:3])
                if t % 2 == 0:
                    nc.scalar.activation(hs[t][:], xbs[t][:], Act.Identity,
                                         bias=ab_t[:, 1:2], scale=ab_t[:, 0:1])
                else:
                    nc.vector.tensor_scalar(hs[t][:], xbs[t][:],
                                            ab_t[:, 0:1], ab_t[:, 1:2],
                                            op0=Alu.mult, op1=Alu.add)
                # wave A: q0 (ot=0) and k0 (ot=4) ride the fresh h tile
                for pq, ot in ((pqq, 0), (pqk, 4)):
                    for half in range(2):
                        nc.tensor.matmul(
                            pq[:, half * 512:(half + 1) * 512],
                            wts[t][:, ot * 128:(ot + 1) * 128],
                            hs[t][:, half * 512:(half + 1) * 512],
                            start=(t == 0), stop=(t == NT - 1))

            qks = [None] * NO_QK
            vs = [None] * NO_QK
            all_pts = [[] for _ in range(NUM_HEADS)]
            attns = [None] * NUM_HEADS

            def emit_bias(ot, pq):
                # q tiles bias on DVE, k tiles on ACT: the pair finishes
                # in parallel so ST can start one bias-latency after qkv.
                # (GpSimd has no PSUM access, so it can't take these.)
                qk_t = pp.tile([128, N], bf16, tag=f"qk{ot}", name=f"qk{ot}")
                if ot < NUM_HEADS:
                    nc.vector.tensor_scalar_add(qk_t[:], pq[:],
                                                bqk[:, ot:ot + 1])
                else:
                    nc.scalar.activation(qk_t[:], pq[:], Act.Identity,
                                         bias=bqk[:, ot:ot + 1])
                qks[ot] = qk_t

            def emit_qkv(ot):
                pq = pm.tile([128, N], f32, tag="acc", name=f"pq{ot}")
                for t in range(NT):
                    for half in range(2):
                        nc.tensor.matmul(
                            pq[:, half * 512:(half + 1) * 512],
                            wts[t][:, ot * 128:(ot + 1) * 128],
                            hs[t][:, half * 512:(half + 1) * 512],
                            start=(t == 0), stop=(t == NT - 1))
                emit_bias(ot, pq)

            def emit_v(nt):
                pv_ = pm.tile([128, N], f32, tag="acc", name=f"pvv{nt}")
                for t in range(NT):
                    nc.tensor.matmul(
                        pv_[:, 0:512],
                        hs[t][:, nt * 128:(nt + 1) * 128],
                        wts[t][:, 2 * C:3 * C],
                        start=(t == 0), stop=(t == NT - 1))
                v_t = pp.tile([128, 512], bf16, tag=f"v{nt}", name=f"v{nt}")
                nc.vector.tensor_copy(v_t[:], pv_[:, 0:512])
                vs[nt] = v_t

            def emit_st(h, jts=None):
                qT = qks[h]
                kT = qks[NUM_HEADS + h]
                if jts is None:
                    jts = range(NO_QK)
                pts = all_pts[h]
                for jt in jts:
                    pst = pm.tile([128, N], f32, tag="ps", name=f"pst{h}_{jt}")
                    for half in range(2):
                        nc.tensor.matmul(
                            pst[:, half * 512:(half + 1) * 512],
                            kT[:, jt * 128:(jt + 1) * 128],
                            qT[:, half * 512:(half + 1) * 512],
                            start=True, stop=True)
                    pt_jt = p_pt.tile([128, N], bf16, tag="pt",
                                      name=f"pt{h}_{jt}")
                    nc.scalar.activation(pt_jt[:], pst[:], Act.Exp, scale=SCALE)
                    pts.append(pt_jt)
                all_pts[h] = pts

            def emit_tree(h):
                pts = all_pts[h]
                # pairwise row-sum tree, all in bf16 (2x DVE mode). The two
                # leading level-1 adds are SBUF->SBUF so they can ride the
                # otherwise-idle GpSimd engine (except for the last head,
                # where GpSimd's slower adds would stretch the tail).
                l1 = nc.gpsimd if h < NUM_HEADS - 1 else nc.vector
                u01 = p_rs.tile([128, N], bf16, tag="rs1", name=f"u01_{h}")
                l1.tensor_add(u01[:], pts[0][:], pts[1][:])
                u23 = p_rs.tile([128, N], bf16, tag="rs1", name=f"u23_{h}")
                l1.tensor_add(u23[:], pts[2][:], pts[3][:])
                u45 = p_rs.tile([128, N], bf16, tag="rs1", name=f"u45_{h}")
                nc.vector.tensor_add(u45[:], pts[4][:], pts[5][:])
                u67 = p_rs.tile([128, N], bf16, tag="rs1", name=f"u67_{h}")
                nc.vector.tensor_add(u67[:], pts[6][:], pts[7][:])
                u0123 = p_rs.tile([128, N], bf16, tag="rs2", name=f"u0123_{h}")
                nc.vector.tensor_add(u0123[:], u01[:], u23[:])
                u4567 = p_rs.tile([128, N], bf16, tag="rs2", name=f"u4567_{h}")
                nc.vector.tensor_add(u4567[:], u45[:], u67[:])
                uallb = p_rs.tile([128, N], bf16, tag="rs2", name=f"uallb_{h}")
                nc.vector.tensor_add(uallb[:], u0123[:], u4567[:])
                return uallb

            def emit_pv_st(hp, hn, uallb):
                # interleave PV of head hp with ST/exp of head hn at jt
                # granularity: ACT's exp stream stays fed while PE does PV
                pts = all_pts[hp] if hp is not None else None
                ppv = None
                if hp is not None:
                    ppv = pm.tile([128, N], f32, tag="acc", name=f"ppv{hp}")
                prs = None
                for jt in range(NO_QK):
                    if hn is not None:
                        emit_st(hn, [jt])
                    if hp is not None and jt == NO_QK - 1:
                        # rowsum before the last PV step: reciprocal +
                        # broadcast overlap the remaining PV matmuls
                        prs = pm.tile([128, N], f32, tag="acc",
                                      name=f"prs{hp}")
                        for half in range(2):
                            nc.tensor.matmul(
                                prs[0:1, half * 512:(half + 1) * 512],
                                ones_r[:],
                                uallb[:, half * 512:(half + 1) * 512],
                                start=True, stop=True)
                    if hp is not None:
                        for half in range(2):
                            nc.tensor.matmul(
                                ppv[:, half * 512:(half + 1) * 512],
                                vs[jt][:, hp * 128:(hp + 1) * 128],
                                pts[jt][:, half * 512:(half + 1) * 512],
                                start=(jt == 0), stop=(jt == NO_QK - 1))
                if hp is None:
                    return
                rr = ps.tile([1, N], f32, tag="rr", bufs=2, name=f"rr{hp}")
                nc.vector.reciprocal_approx_fast(rr[:], prs[0:1, :])
                rb = ps.tile([128, N], f32, tag="rb", bufs=2, name=f"rb{hp}")
                nc.gpsimd.partition_broadcast(rb[:], rr[:])
                attn_h = pp.tile([128, N], bf16, tag=f"attn{hp}",
                                 name=f"attn{hp}")
                nc.vector.tensor_mul(attn_h[:], ppv[:], rb[:])
                attns[hp] = attn_h

            pprs = [None] * NT

            def emit_proj_mm(ot, h):
                for half in range(2):
                    nc.tensor.matmul(
                        pprs[ot][:, half * 512:(half + 1) * 512],
                        pwts[h][:, ot * 128:(ot + 1) * 128],
                        attns[h][:, half * 512:(half + 1) * 512],
                        start=(h == 0), stop=(h == NUM_HEADS - 1))

            def emit_out(ot):
                o_t = p_out.tile([128, N], f32, tag="out", name=f"o{ot}")
                nc.vector.scalar_tensor_tensor(o_t[:], pprs[ot][:],
                                               beff[:, ot:ot + 1], xbs[ot][:],
                                               op0=Alu.add, op1=Alu.add)
                eng = nc.scalar if ot % 2 == 1 else nc.sync
                eng.dma_start(d_out[ot * 128:(ot + 1) * 128, :], o_t[:])

            # interleaved schedule: head 0's qkv came from wave A; remaining
            # qkv pairs + V feed heads as their dependencies resolve.
            emit_bias(0, pqq)
            emit_bias(4, pqk)
            emit_st(0)
            emit_qkv(1); emit_qkv(5)
            emit_st(1)
            emit_qkv(2); emit_qkv(6)
            emit_qkv(3); emit_qkv(7)
            for nt in range(NO_QK):
                emit_v(nt)
            u0 = emit_tree(0)
            emit_pv_st(0, 2, u0)
            u1 = emit_tree(1)
            emit_pv_st(1, 3, u1)
            u2 = emit_tree(2)
            emit_pv_st(2, None, u2)
            # ---- proj wave 1 (out tiles 0,1) accumulates heads as they land
            pprs[0] = pm.tile([128, N], f32, tag="ps", name="ppr0")
            pprs[1] = pm.tile([128, N], f32, tag="ps", name="ppr1")
            for h in range(2):
                emit_proj_mm(0, h)
                emit_proj_mm(1, h)
            u3 = emit_tree(3)
            emit_pv_st(3, None, u3)
            for h in range(2, NUM_HEADS):
                emit_proj_mm(0, h)
                emit_proj_mm(1, h)
            emit_out(0)
            emit_out(1)
            # ---- proj wave 2 (out tiles 2,3)
            pprs[2] = pm.tile([128, N], f32, tag="ps", name="ppr2")
            pprs[3] = pm.tile([128, N], f32, tag="ps", name="ppr3")
            for h in range(NUM_HEADS):
                emit_proj_mm(2, h)
                emit_proj_mm(3, h)
            emit_out(2)
            emit_out(3)

    nc.compile()
    return nc


def make_in_maps(x, norm_w, norm_b, qkv_w, qkv_b, proj_w, proj_b):
    x = np.asarray(x, dtype=np.float32)
    qkv_w = np.asarray(qkv_w, dtype=np.float32)
    qkv_b = np.asarray(qkv_b, dtype=np.float32)
    proj_w = np.asarray(proj_w, dtype=np.float32)
    proj_b = np.asarray(proj_b, dtype=np.float32)

    import ml_dtypes
    wt = np.ascontiguousarray(qkv_w.T).astype(ml_dtypes.bfloat16)   # [C, 3C]
    pwt = np.ascontiguousarray(proj_w.T).astype(ml_dtypes.bfloat16)  # [C, C]
    b_eff = (proj_b + proj_w @ qkv_b[2 * C:3 * C]).astype(np.float32)
    bias_qk = np.ascontiguousarray(qkv_b[:2 * C])

    p = np.arange(128)
    sel = (p[:, None] // GROUP_CH == np.arange(8)[None, :]).astype(np.float32)
    selT = np.ascontiguousarray(sel.T)

    xs = x.reshape(B, C, N)
    cvec = np.zeros((128, 28), np.float32)
    cvec[:, 0:4] = np.asarray(norm_w, np.float32).reshape(4, 128).T
    cvec[:, 4:8] = np.asarray(norm_b, np.float32).reshape(4, 128).T
    cvec[:, 8:16] = bias_qk.reshape(8, 128).T
    cvec[:, 16:20] = b_eff.reshape(4, 128).T
    cvec[:, 20:28] = sel
    common = {
        "qkv_wt": wt, "proj_wt": pwt, "cvec": cvec, "selT": selT,
        "ones": np.ones((128, 1), ml_dtypes.bfloat16),
    }
    return [dict(common,
                 xb=np.ascontiguousarray(xs[i]).astype(ml_dtypes.bfloat16))
            for i in range(B)]


def run(inputs, trace=False, tmpdir=None):
    from concourse.bass_utils import run_bass_kernel_spmd
    nc = build_bass()
    in_maps = make_in_maps(**inputs)
    res = run_bass_kernel_spmd(nc, in_maps, core_ids=list(range(N_CORES)),
                               trace=trace, tmpdir=tmpdir)
    out = np.stack([res.results[i]["out"] for i in range(N_CORES)])
    return out.reshape(B, C, H, W).astype(np.float32), res


def kernel(**inputs):
    out, _ = run(inputs, trace=False)
    return out


# revision 27
# speedup vs baseline: 1.0927x; 1.0927x over previous
"""AttentionBlock (GroupNorm -> 1x1 qkv -> 4-head attention -> 1x1 proj -> residual)
on 8 trn2 NeuronCores, data-parallel over the batch dim (B=8, one element/core).

Layout per core: channel-major [C=512, N=1024] as 4 SBUF tiles of [128, 1024].
V is computed spatial-major directly from the qkv matmul so attention needs no
transposes:
  ST[j,i] = sum_d k[d,j] q[d,i]   (K-tile stationary)
  p~T[j,i] = exp(scale*ST)        (ScalarE, PSUM->SBUF)
  rowsum[i] = ones^T @ p~T        (PE)
  PV[d,i] = sum_j v_sp[j,d] p~T[j,i]   -> channel-major attention output
  out = PV * (1/rowsum broadcast)      (softmax divide deferred past PV)
v-bias is folded into proj bias on the host (softmax rows sum to 1).

v2 schedule: residual taken from the bf16 x copy (no f32 x load), GN rstd via
ln/exp (one ACT table for the whole kernel), q0/k0 qkv matmuls wave-interleaved
with groupnorm, qkv bias adds on DVE (q) / GpSimd (k), v casts on GpSimd,
all-bf16 softmax reduction tree, proj in two PSUM waves starting as heads
complete.
"""

import numpy as np

B, C, H, W = 8, 512, 32, 32
N = H * W  # 1024
NUM_HEADS = 4
HEAD_DIM = C // NUM_HEADS  # 128
NUM_GROUPS = 32
GROUP_CH = C // NUM_GROUPS  # 16
EPS = 1e-5
NT = C // 128  # 4 channel tiles
NO_QK = 8  # q,k output tiles (1024 channels)
SCALE = 1.0 / float(np.sqrt(HEAD_DIM))
N_CORES = 8


def build_bass():
    import concourse.bacc as bacc
    import concourse.tile as tile
    from concourse import mybir

    f32 = mybir.dt.float32
    bf16 = mybir.dt.bfloat16
    Act = mybir.ActivationFunctionType
    Alu = mybir.AluOpType
    Ax = mybir.AxisListType

    nc = bacc.Bacc("TRN2", target_bir_lowering=False, debug=False,
                   num_devices=N_CORES)

    d_xb = nc.declare_dram_parameter("xb", [C, N], bf16, isOutput=False)
    d_wqk = nc.declare_dram_parameter("qk_wt", [C, 2 * C], bf16, isOutput=False)
    d_wv = nc.declare_dram_parameter("v_wt", [C, C], bf16, isOutput=False)
    d_pwt = nc.declare_dram_parameter("proj_wt", [C, C], bf16, isOutput=False)
    d_cv = nc.declare_dram_parameter("cvec", [128, 28], f32, isOutput=False)
    d_selT = nc.declare_dram_parameter("selT", [8, 128], f32, isOutput=False)
    d_ones = nc.declare_dram_parameter("ones", [128, 1], bf16, isOutput=False)
    d_out = nc.declare_dram_parameter("out", [C, N], bf16, isOutput=True)

    with tile.TileContext(nc) as tc:
        with (
            tc.tile_pool(name="persist", bufs=1) as pp,
            tc.tile_pool(name="pt", bufs=20) as p_pt,
            tc.tile_pool(name="rsum", bufs=5) as p_rs,
            tc.tile_pool(name="outp", bufs=2) as p_out,
            tc.tile_pool(name="small", bufs=1) as ps,
            tc.tile_pool(name="psum", bufs=2, space="PSUM") as pm,
        ):
            # ---- tiny constants + ACT table warm (exp then ln narrows the
            # activation table to natural_log_exp_and_others, which also
            # serves Square/Identity: one table load for the whole kernel,
            # during the DMA window).
            warm = ps.tile([128, 512], bf16, tag="warm", name="warm")
            nc.vector.memset(warm[:], 0.5)
            epsv = ps.tile([8, 2], f32, tag="epsv", name="epsv")
            nc.vector.memset(epsv[:], EPS)

            cvec = ps.tile([128, 28], f32, tag="cvec", name="cvec")
            gam, bet, bqk, beff = (cvec[:, 0:4], cvec[:, 4:8],
                                   cvec[:, 8:16], cvec[:, 16:20])
            sel = cvec[:, 20:28]
            selT = ps.tile([8, 128], f32, tag="selT", name="selT")
            ones_r = ps.tile([128, 1], bf16, tag="ones_r", name="ones_r")

            # ---- bulk loads. The gpsimd SWDGE queue stripes across DMA
            # engines (~160 GB/s) while each HWDGE queue is pinned to one
            # engine (~15-20 GB/s), so everything on the GN->qkv critical
            # path goes through SWDGE in dependency order; the slow queues
            # carry xb3 (split in half), proj weights, and the outputs.
            xbs, hs, wqks, wvs, pwts = [], [], [], [], []
            for t in range(NT):
                xb_t = pp.tile([128, N], bf16, tag=f"xb{t}", name=f"xb{t}")
                xbs.append(xb_t)
            for t in range(NT):
                wqk_t = pp.tile([128, 2 * C], bf16, tag=f"wqk{t}",
                                name=f"wqk{t}")
                wqks.append(wqk_t)
            for t in range(NT):
                wv_t = pp.tile([128, C], bf16, tag=f"wv{t}", name=f"wv{t}")
                wvs.append(wv_t)
            for t in range(3):
                nc.gpsimd.dma_start(xbs[t][:], d_xb[t * 128:(t + 1) * 128, :])
                nc.gpsimd.dma_start(wqks[t][:],
                                    d_wqk[t * 128:(t + 1) * 128, :])
            nc.gpsimd.dma_start(wqks[3][:], d_wqk[384:512, :])
            nc.sync.dma_start(selT[:], d_selT[:, :])
            nc.sync.dma_start(cvec[:], d_cv[:, :])
            nc.sync.dma_start(xbs[3][0:64, :], d_xb[384:448, :])
            nc.scalar.dma_start(ones_r[:], d_ones[:, :])
            nc.scalar.dma_start(xbs[3][64:128, :], d_xb[448:512, :])
            for t in range(NT):
                nc.gpsimd.dma_start(wvs[t][:], d_wv[t * 128:(t + 1) * 128, :])
            for t in range(NT):
                pwt_t = pp.tile([128, C], bf16, tag=f"pwt{t}", name=f"pwt{t}")
                eng = nc.sync if t < 2 else nc.scalar
                eng.dma_start(pwt_t[:], d_pwt[t * 128:(t + 1) * 128, :])
                pwts.append(pwt_t)

            # PE warm-up: junk matmul chain (never read) holds the PE p-state
            # up while the first xb tiles stream in.
            junk = pm.tile([128, N], f32, tag="acc", name="junk")

            def junk_mm(n, first=False, last=False):
                for j in range(n):
                    nc.tensor.matmul(junk[0:128, 0:512], warm[:, 0:128],
                                     warm[:, 0:512],
                                     start=(first and j == 0),
                                     stop=(last and j == n - 1),
                                     skip_group_check=True)

            junk_mm(3, first=True, last=True)

            # q0/k0 accumulate across GN tiles (wave A)
            pqq = pm.tile([128, N], f32, tag="acc", name="pqq")
            pqk = pm.tile([128, N], f32, tag="acc", name="pqk")

            # ---- group norm per-tile (groups never cross 128-ch tiles),
            # with the head-0 qkv matmuls riding along as h tiles appear.
            for t in range(NT):
                h_t = pp.tile([128, N], bf16, tag=f"h{t}", name=f"h{t}")
                hs.append(h_t)
            for t in range(NT):
                st_t = ps.tile([128, 2], f32, tag=f"st{t}", name=f"st{t}")
                nc.vector.reduce_sum(st_t[:, 0:1], xbs[t][:], axis=Ax.X)
                nc.scalar.activation(hs[t][:], xbs[t][:], Act.Square,
                                     accum_out=st_t[:, 1:2])
                psg = pm.tile([128, N], f32, tag="ps", name=f"psg{t}")
                nc.tensor.matmul(psg[0:8, 0:2], sel, st_t[:, 0:2],
                                 start=True, stop=True)
                inv_n = 1.0 / float(GROUP_CH * N)
                msr = ps.tile([8, 4], f32, tag=f"msr{t}", name=f"msr{t}")
                nc.scalar.mul(msr[:, 0:1], psg[0:8, 0:1], inv_n)
                nc.scalar.square(msr[:, 3:4], msr[:, 0:1])
                nc.vector.scalar_tensor_tensor(msr[:, 2:3], psg[0:8, 1:2],
                                               inv_n, msr[:, 3:4],
                                               op0=Alu.mult, op1=Alu.subtract)
                nc.scalar.activation(msr[:, 3:4], msr[:, 2:3], Act.Sqrt,
                                     bias=epsv[:, 0:1])
                nc.vector.reciprocal(msr[:, 1:2], msr[:, 3:4])
                pse = pm.tile([128, N], f32, tag="ps", name=f"pse{t}")
                nc.tensor.matmul(pse[:, 0:2], selT[:], msr[:, 0:2],
                                 start=True, stop=True)
                ab_t = ps.tile([128, 3], f32, tag=f"ab{t}", name=f"ab{t}")
                nc.vector.tensor_mul(ab_t[:, 0:1], gam[:, t:t + 1], pse[:, 1:2])
                nc.vector.tensor_mul(ab_t[:, 2:3], pse[:, 0:1], ab_t[:, 0:1])
                nc.vector.tensor_sub(ab_t[:, 1:2], bet[:, t:t + 1], ab_t[:, 2:3])
                if t % 2 == 0:
                    nc.scalar.activation(hs[t][:], xbs[t][:], Act.Identity,
                                         bias=ab_t[:, 1:2], scale=ab_t[:, 0:1])
                else:
                    nc.vector.tensor_scalar(hs[t][:], xbs[t][:],
                                            ab_t[:, 0:1], ab_t[:, 1:2],
                                            op0=Alu.mult, op1=Alu.add)
                # wave A: q0 (ot=0) and k0 (ot=4) ride the fresh h tile
                for pq, ot in ((pqq, 0), (pqk, 4)):
                    for half in range(2):
                        nc.tensor.matmul(
                            pq[:, half * 512:(half + 1) * 512],
                            wqks[t][:, ot * 128:(ot + 1) * 128],
                            hs[t][:, half * 512:(half + 1) * 512],
                            start=(t == 0), stop=(t == NT - 1))

            qks = [None] * NO_QK
            vs = [None] * NO_QK
            all_pts = [[] for _ in range(NUM_HEADS)]
            attns = [None] * NUM_HEADS

            def emit_bias(ot, pq):
                # q tiles bias on DVE, k tiles on ACT: the pair finishes
                # in parallel so ST can start one bias-latency after qkv.
                # (GpSimd has no PSUM access, so it can't take these.)
                qk_t = pp.tile([128, N], bf16, tag=f"qk{ot}", name=f"qk{ot}")
                if ot < NUM_HEADS:
                    nc.vector.tensor_scalar_add(qk_t[:], pq[:],
                                                bqk[:, ot:ot + 1])
                else:
                    nc.scalar.activation(qk_t[:], pq[:], Act.Identity,
                                         bias=bqk[:, ot:ot + 1])
                qks[ot] = qk_t

            def emit_qkv(ot):
                pq = pm.tile([128, N], f32, tag="acc", name=f"pq{ot}")
                for t in range(NT):
                    for half in range(2):
                        nc.tensor.matmul(
                            pq[:, half * 512:(half + 1) * 512],
                            wqks[t][:, ot * 128:(ot + 1) * 128],
                            hs[t][:, half * 512:(half + 1) * 512],
                            start=(t == 0), stop=(t == NT - 1))
                emit_bias(ot, pq)

            def emit_v(nt):
                pv_ = pm.tile([128, N], f32, tag="acc", name=f"pvv{nt}")
                for t in range(NT):
                    nc.tensor.matmul(
                        pv_[:, 0:512],
                        hs[t][:, nt * 128:(nt + 1) * 128],
                        wvs[t][:, 0:512],
                        start=(t == 0), stop=(t == NT - 1))
                v_t = pp.tile([128, 512], bf16, tag=f"v{nt}", name=f"v{nt}")
                nc.vector.tensor_copy(v_t[:], pv_[:, 0:512])
                vs[nt] = v_t

            def emit_st(h, jts=None):
                qT = qks[h]
                kT = qks[NUM_HEADS + h]
                if jts is None:
                    jts = range(NO_QK)
                pts = all_pts[h]
                for jt in jts:
                    pst = pm.tile([128, N], f32, tag="ps", name=f"pst{h}_{jt}")
                    for half in range(2):
                        nc.tensor.matmul(
                            pst[:, half * 512:(half + 1) * 512],
                            kT[:, jt * 128:(jt + 1) * 128],
                            qT[:, half * 512:(half + 1) * 512],
                            start=True, stop=True)
                    pt_jt = p_pt.tile([128, N], bf16, tag="pt",
                                      name=f"pt{h}_{jt}")
                    nc.scalar.activation(pt_jt[:], pst[:], Act.Exp, scale=SCALE)
                    pts.append(pt_jt)
                all_pts[h] = pts

            def emit_tree(h, defer7=False):
                pts = all_pts[h]
                # pairwise row-sum tree, all in bf16 (2x DVE mode). The two
                # leading level-1 adds are SBUF->SBUF so they can ride the
                # otherwise-idle GpSimd engine (except for the last head,
                # where GpSimd's slower adds would stretch the tail).
                # defer7: leave pt[7] out of the tree (summed by a second
                # accumulating ones-matmul), so the reciprocal/broadcast
                # chain starts one exp earlier on the critical tail.
                l1 = nc.gpsimd if h < NUM_HEADS - 1 else nc.vector
                u01 = p_rs.tile([128, N], bf16, tag="rs1", name=f"u01_{h}")
                l1.tensor_add(u01[:], pts[0][:], pts[1][:])
                u23 = p_rs.tile([128, N], bf16, tag="rs1", name=f"u23_{h}")
                l1.tensor_add(u23[:], pts[2][:], pts[3][:])
                u45 = p_rs.tile([128, N], bf16, tag="rs1", name=f"u45_{h}")
                nc.vector.tensor_add(u45[:], pts[4][:], pts[5][:])
                u0123 = p_rs.tile([128, N], bf16, tag="rs2", name=f"u0123_{h}")
                nc.vector.tensor_add(u0123[:], u01[:], u23[:])
                if defer7:
                    u456 = p_rs.tile([128, N], bf16, tag="rs1",
                                     name=f"u456_{h}")
                    nc.vector.tensor_add(u456[:], u45[:], pts[6][:])
                    uallb = p_rs.tile([128, N], bf16, tag="rs2",
                                      name=f"uallb_{h}")
                    nc.vector.tensor_add(uallb[:], u0123[:], u456[:])
                    return uallb, pts[7]
                u67 = p_rs.tile([128, N], bf16, tag="rs1", name=f"u67_{h}")
                nc.vector.tensor_add(u67[:], pts[6][:], pts[7][:])
                u4567 = p_rs.tile([128, N], bf16, tag="rs2", name=f"u4567_{h}")
                nc.vector.tensor_add(u4567[:], u45[:], u67[:])
                uallb = p_rs.tile([128, N], bf16, tag="rs2", name=f"uallb_{h}")
                nc.vector.tensor_add(uallb[:], u0123[:], u4567[:])
                return uallb, None

            def emit_pv_st(hp, hn, uallb, upt7=None):
                # interleave PV of head hp with ST/exp of head hn at jt
                # granularity: ACT's exp stream stays fed while PE does PV
                pts = all_pts[hp] if hp is not None else None
                ppv = None
                if hp is not None:
                    ppv = pm.tile([128, N], f32, tag="acc", name=f"ppv{hp}")
                prs = None
                for jt in range(NO_QK):
                    if hn is not None:
                        emit_st(hn, [jt])
                    if hp is not None and jt == NO_QK - 1:
                        # rowsum before the last PV step: reciprocal +
                        # broadcast overlap the remaining PV matmuls
                        prs = pm.tile([128, N], f32, tag="acc",
                                      name=f"prs{hp}")
                        srcs = [uallb] if upt7 is None else [uallb, upt7]
                        for half in range(2):
                            for si, src in enumerate(srcs):
                                nc.tensor.matmul(
                                    prs[0:1, half * 512:(half + 1) * 512],
                                    ones_r[:],
                                    src[:, half * 512:(half + 1) * 512],
                                    start=(si == 0), stop=(si == len(srcs) - 1))
                    if hp is not None:
                        for half in range(2):
                            nc.tensor.matmul(
                                ppv[:, half * 512:(half + 1) * 512],
                                vs[jt][:, hp * 128:(hp + 1) * 128],
                                pts[jt][:, half * 512:(half + 1) * 512],
                                start=(jt == 0), stop=(jt == NO_QK - 1))
                if hp is None:
                    return
                rr = ps.tile([1, N], f32, tag="rr", bufs=2, name=f"rr{hp}")
                nc.vector.reciprocal_approx_fast(rr[:], prs[0:1, :])
                rb = ps.tile([128, N], f32, tag="rb", bufs=2, name=f"rb{hp}")
                nc.gpsimd.partition_broadcast(rb[:], rr[:])
                attn_h = pp.tile([128, N], bf16, tag=f"attn{hp}",
                                 name=f"attn{hp}")
                nc.vector.tensor_mul(attn_h[:], ppv[:], rb[:])
                attns[hp] = attn_h

            pprs = [None] * NT

            def emit_proj_mm(ot, h):
                for half in range(2):
                    nc.tensor.matmul(
                        pprs[ot][:, half * 512:(half + 1) * 512],
                        pwts[h][:, ot * 128:(ot + 1) * 128],
                        attns[h][:, half * 512:(half + 1) * 512],
                        start=(h == 0), stop=(h == NUM_HEADS - 1))

            def emit_out(ot):
                o_t = p_out.tile([128, N], bf16, tag="out", name=f"o{ot}")
                nc.vector.scalar_tensor_tensor(o_t[:], pprs[ot][:],
                                               beff[:, ot:ot + 1], xbs[ot][:],
                                               op0=Alu.add, op1=Alu.add)
                eng = nc.scalar if ot % 2 == 1 else nc.sync
                eng.dma_start(d_out[ot * 128:(ot + 1) * 128, :], o_t[:])

            # interleaved schedule: head 0's qkv came from wave A; remaining
            # qkv pairs + V feed heads as their dependencies resolve.
            dum = ps.tile([1, 1], f32, tag="dum", name="dum")
            nc.scalar.activation(dum[:], epsv[0:1, 0:1], Act.Exp)
            emit_bias(0, pqq)
            emit_bias(4, pqk)
            emit_st(0)
            emit_qkv(1); emit_qkv(5)
            emit_st(1)
            emit_qkv(2); emit_qkv(6)
            emit_qkv(3); emit_qkv(7)
            for nt in range(NO_QK):
                emit_v(nt)
            u0, _ = emit_tree(0)
            emit_pv_st(0, 2, u0)
            u1, _ = emit_tree(1)
            emit_pv_st(1, 3, u1)
            u2, _ = emit_tree(2)
            emit_pv_st(2, None, u2)
            # ---- proj wave 1 (out tiles 0,1) accumulates heads as they land
            pprs[0] = pm.tile([128, N], f32, tag="ps", name="ppr0")
            pprs[1] = pm.tile([128, N], f32, tag="ps", name="ppr1")
            for h in range(2):
                emit_proj_mm(0, h)
                emit_proj_mm(1, h)
            u3, upt7 = emit_tree(3, defer7=True)
            emit_pv_st(3, None, u3, upt7)
            for h in range(2, NUM_HEADS):
                emit_proj_mm(0, h)
                emit_proj_mm(1, h)
            emit_out(0)
            emit_out(1)
            # ---- proj wave 2 (out tiles 2,3)
            pprs[2] = pm.tile([128, N], f32, tag="ps", name="ppr2")
            pprs[3] = pm.tile([128, N], f32, tag="ps", name="ppr3")
            for h in range(NUM_HEADS):
                emit_proj_mm(2, h)
                emit_proj_mm(3, h)
            emit_out(2)
            emit_out(3)

    nc.compile()
    return nc


def make_in_maps(x, norm_w, norm_b, qkv_w, qkv_b, proj_w, proj_b):
    x = np.asarray(x, dtype=np.float32)
    qkv_w = np.asarray(qkv_w, dtype=np.float32)
    qkv_b = np.asarray(qkv_b, dtype=np.float32)
    proj_w = np.asarray(proj_w, dtype=np.float32)
    proj_b = np.asarray(proj_b, dtype=np.float32)

    import ml_dtypes
    wt = np.ascontiguousarray(qkv_w.T).astype(ml_dtypes.bfloat16)   # [C, 3C]
    wqk = np.ascontiguousarray(wt[:, 0:2 * C])
    wv = np.ascontiguousarray(wt[:, 2 * C:3 * C])
    pwt = np.ascontiguousarray(proj_w.T).astype(ml_dtypes.bfloat16)  # [C, C]
    b_eff = (proj_b + proj_w @ qkv_b[2 * C:3 * C]).astype(np.float32)
    bias_qk = np.ascontiguousarray(qkv_b[:2 * C])

    p = np.arange(128)
    sel = (p[:, None] // GROUP_CH == np.arange(8)[None, :]).astype(np.float32)
    selT = np.ascontiguousarray(sel.T)

    xs = x.reshape(B, C, N)
    cvec = np.zeros((128, 28), np.float32)
    cvec[:, 0:4] = np.asarray(norm_w, np.float32).reshape(4, 128).T
    cvec[:, 4:8] = np.asarray(norm_b, np.float32).reshape(4, 128).T
    cvec[:, 8:16] = bias_qk.reshape(8, 128).T
    cvec[:, 16:20] = b_eff.reshape(4, 128).T
    cvec[:, 20:28] = sel
    common = {
        "qk_wt": wqk, "v_wt": wv, "proj_wt": pwt, "cvec": cvec, "selT": selT,
        "ones": np.ones((128, 1), ml_dtypes.bfloat16),
    }
    return [dict(common,
                 xb=np.ascontiguousarray(xs[i]).astype(ml_dtypes.bfloat16))
            for i in range(B)]


def run(inputs, trace=False, tmpdir=None):
    from concourse.bass_utils import run_bass_kernel_spmd
    nc = build_bass()
    in_maps = make_in_maps(**inputs)
    res = run_bass_kernel_spmd(nc, in_maps, core_ids=list(range(N_CORES)),
                               trace=trace, tmpdir=tmpdir)
    out = np.stack([res.results[i]["out"] for i in range(N_CORES)])
    return out.reshape(B, C, H, W).astype(np.float32), res


def kernel(**inputs):
    out, _ = run(inputs, trace=False)
    return out
